# revision 17
# baseline (speedup 1.0000x reference)
"""Trainium2 Bass kernel for the MichaelsRNN forward pass.

Reference math (per time step t, per batch element b):
    recur = r @ J.T
    inp   = image.T @ I.T + hold.T * S.T
    pre   = 0.9*x + 0.1*(recur + inp + Bb.T)     # Euler step dt/tau = 1/10
    out   = retanh(pre) = max(tanh(pre), 0)
    y     = out[:, :100] @ fc_w.T + fc_b
    carry = (pre, out)

Sharding: data-parallel over batch. B=1024 over 8 cores = 128/core, in
two phase-shifted half-batches of 64 (PE runs half B's matmul group
while ACT/DVE run half A's elementwise).

Wall-clock here is dominated by the axon tunnel: ~40 MB/s TOTAL,
shared between directions and across all 8 devices (measured: no
duplex win, no multi-stream win, no D2H compression). Device exec is
~free (<2% of wall). So the kernel is engineered to minimize wire
bytes; precision is the scarce resource (rel-err gate 2e-2, error
amplified by the recurrent J with spectral radius ~1.2):

  - weights: f32 (PE f32 matmul is free here; bf16 weights alone cost
    0.6% rel err). Uploaded as ONE sharded copy (122x128 slice per
    core) and AllGather'd on device (0.5MB instead of 4MB x8).
  - din: int8 with a cubic companding curve (decode on device is a
    2-op poly; finer steps near 0 cut the 0.9% uniform-int8 error to
    ~0.6%). 10.75MB.
  - y: closed-loop DPCM, per-(step,row) scale: the device keeps the
    decoder state yhat, quantizes d = y_t - yhat to Y_BITS symbols
    with scale s = rowmax|d|/qm (shipped bf16), packs symbols into
    bytes (6-bit: 4 vals -> 3B = 19.2MB; 5-bit: 8 vals -> 5B = 16MB),
    vs 25.6MB for direct int8 -- and with LESS quantization error
    (deltas are ~8x smaller than y).
    Host decode = unpack + cumsum(q*s); it need not be bit-exact with
    the device's yhat (f32 drift ~1e-7 << budget).

Host-side (1 CPU core!) work is pipelined under the wire: din is
companded/packed per chunk while the previous chunk uploads; y shards
decode while the next shard downloads.

Per half-step, ONE PSUM accumulation group in one bank:
    9x J matmul      lhsT[122,100]=[0.1J[m,k].T ; k==0?[0.1I;0.1S;0.1Bb]:0]
                     rhs=rd_h[0:122, k]  (k0m0 opens the group)
    1x fc matmul     lhsT=[fc_w.T;0] [122,50] rhs=rd_h = y of t-1 (stop)
The Euler step pre' = 0.9*pre + psum runs on DVE; r = retanh on
ACT(tanh)+DVE(max). y_{t-1} is computed inside step t's group (its
input r_{t-1} is still live), so it costs no extra PSUM group.

State per half (ping-pong on step parity):
    rd_{h,p} [122, 192] f32: rows 0:100 = r; rows 100:121 of module
        slice 0 = the step's companded [image;hold] (staged 2 steps
        ahead); row 121 slice 0 = ones (drives Bb); rows 100:122 of
        slices 1,2 = zeros (meet zero weights only).
"""

import numpy as np
import ml_dtypes

import concourse.bass as bass  # noqa: F401
import concourse.tile as tile
from concourse import bacc, mybir

NPM = 100
NMOD = 3
NN = 300
NF = 20
OUT = 50
T = 500
B = 1024
N_CORES = 8
BS = B // N_CORES      # 128 batch per core
NH = 2                 # phase-shifted half-batches
HB = BS // NH          # 64
HFREE = NMOD * HB      # 192
KDATA = NF + 1         # 21 data rows on the wire (image, hold)
KD = KDATA + 1         # 22 data rows in SBUF (plus ones)
KJ = NPM + KD          # 122

Y_BITS = 5             # 6: pack 4 vals->3B; 5: pack 8 vals->5B
D_CUBIC_W = 0.45       # din compander: dec(u) = amax*(w*u+(1-w)*u^3); 0 -> uniform
W_MODE = "gather"      # "gather": 1 sharded copy + on-device AllGather; "repl": x8
N_UP_CHUNKS = 2        # din upload pipeline granularity (fewer -> less per-
                       # transfer overhead; prep is LUT-fast so 2 suffices)

# y wire geometry
if Y_BITS == 6:
    YGRP, YGB = 4, 3          # 4 steps -> 3 byte-planes
    QM_EFF = 30.5             # |q| <= 31 incl. reciprocal slop; v=q+31 in [0,62]
    Q_OFF = 31.0
    CH = 20                   # steps per y-out chunk (multiple of YGRP)
else:
    YGRP, YGB = 8, 5
    QM_EFF = 14.5
    Q_OFF = 15.0
    CH = 24

# f32 weight pack, per-core shard is a [KJ, 128] column block; everything
# past col 896 lives in core 7's block so gather-mode DMAs stay simple
WALL_JT = 9 * NPM                  # cols 0:900   jt
WALL_FCT = 904                     # cols 904:954 fct
WALL_ONES = 954                    # cols 954:1018  ones row (row 0)
WALL_C1 = 1018                     # col 1018     compander c1 (rows 0:KDATA)
WALL_C3 = 1019                     # col 1019     compander c3 (rows 0:KDATA)
WALL_FCB = 1020                    # col 1020     fcb (rows 0:OUT)
WALL_X0 = 1021                     # cols 1021:1024  x0 modules
WALL_COLS = 1024
WBLK = WALL_COLS // N_CORES        # 128

_BUILD_CACHE: dict = {}
_RUNNER_CACHE: dict = {}


def _ceil_div(a, b):
    return -(-a // b)


def _ygroups(n_steps):
    return _ceil_div(n_steps, YGRP)


def _build_program(n_steps: int, n_repeat: int = 1):
    """Build + compile the Bass program (value-independent)."""
    f32 = mybir.dt.float32
    bf16 = mybir.dt.bfloat16
    i8 = mybir.dt.int8
    i32 = mybir.dt.int32
    import contextlib

    nc = bacc.Bacc(
        "TRN2", target_bir_lowering=False, debug=False, num_devices=N_CORES
    )

    chunks = _plan_up_chunks(n_steps)
    din_aps = []
    for i, c in enumerate(chunks):
        din_aps.append(
            nc.dram_tensor(f"din{i}", [KDATA, c * BS], i8, kind="ExternalInput").ap()
        )
    if W_MODE == "gather":
        w_ap = nc.dram_tensor("w", [KJ, WBLK], f32, kind="ExternalInput").ap()
    else:
        w_ap = nc.dram_tensor("w", [KJ, WALL_COLS], f32, kind="ExternalInput").ap()
    ng = _ygroups(n_steps)
    y_ap = nc.dram_tensor(
        "y", [OUT, ng * YGB * BS], i8, kind="ExternalOutput"
    ).ap()
    sc_ap = nc.dram_tensor(
        "sc", [OUT, n_steps], bf16, kind="ExternalOutput"
    ).ap()

    ch = min(CH, n_steps)
    cum = np.cumsum([0] + chunks)

    def dslice(t, h):
        i = int(np.searchsorted(cum, t, side="right")) - 1
        off = (t - cum[i]) * BS + h * HB
        return din_aps[i][:, off : off + HB]

    with tile.TileContext(nc) as tc:
        with contextlib.ExitStack() as ctx:
            const_pool = ctx.enter_context(tc.tile_pool(name="const", bufs=1))
            yout_pool = ctx.enter_context(tc.tile_pool(name="yout", bufs=2))
            tmp_pool = ctx.enter_context(tc.tile_pool(name="tmp", bufs=2))
            ps_pool = ctx.enter_context(
                tc.tile_pool(name="ps", bufs=2, space="PSUM")
            )

            # ---- weights: AllGather one sharded copy, or use replicated ----
            if W_MODE == "gather":
                dram_pool = ctx.enter_context(
                    tc.tile_pool(name="dramw", bufs=1, space="DRAM")
                )
                win = dram_pool.tile([KJ, WBLK], f32)
                wg = dram_pool.tile([N_CORES * KJ, WBLK], f32)
                nc.gpsimd.dma_start(win[:], w_ap)
                nc.gpsimd.collective_compute(
                    "AllGather",
                    mybir.AluOpType.bypass,
                    replica_groups=[list(range(N_CORES))],
                    ins=[win[:].opt()],
                    outs=[wg[:].opt()],
                )

                def wall(c0, c1, r0=0, r1=KJ):
                    """list of (dram slice, dst col offset) covering cols c0:c1."""
                    outl = []
                    for c in range(N_CORES):
                        a = max(c0, c * WBLK)
                        b = min(c1, (c + 1) * WBLK)
                        if a < b:
                            outl.append(
                                (
                                    wg[c * KJ + r0 : c * KJ + r1,
                                       a - c * WBLK : b - c * WBLK],
                                    a - c0,
                                    b - a,
                                )
                            )
                    return outl
            else:

                def wall(c0, c1, r0=0, r1=KJ):
                    return [(w_ap[r0:r1, c0:c1], 0, c1 - c0)]

            jt = const_pool.tile([KJ, 9 * NPM], f32)
            for src, off, n in wall(0, 9 * NPM):
                nc.sync.dma_start(jt[:, off : off + n], src)
            fct = const_pool.tile([KJ, OUT], f32)
            for src, off, n in wall(WALL_FCT, WALL_FCT + OUT):
                nc.sync.dma_start(fct[:, off : off + n], src)
            fcb = const_pool.tile([OUT, 1], f32)
            for src, off, n in wall(WALL_FCB, WALL_FCB + 1, 0, OUT):
                nc.sync.dma_start(fcb[:, off : off + n], src)
            x0m = const_pool.tile([NPM, NMOD], f32)
            for src, off, n in wall(WALL_X0, WALL_X0 + NMOD, 0, NPM):
                nc.sync.dma_start(x0m[:, off : off + n], src)

            zeros = const_pool.tile([NPM, HFREE], f32)
            nc.vector.memset(zeros[:], 0.0)
            zf32 = const_pool.tile([NPM, HB], f32)
            nc.vector.memset(zf32[:], 0.0)

            pre_a0 = const_pool.tile([NPM, HFREE], f32)
            pre_a1 = const_pool.tile([NPM, HFREE], f32)
            pre_b0 = const_pool.tile([NPM, HFREE], f32)
            pre_b1 = const_pool.tile([NPM, HFREE], f32)
            pres = [[pre_a0, pre_a1], [pre_b0, pre_b1]]
            rd_a0 = const_pool.tile([KJ, HFREE], f32)
            rd_a1 = const_pool.tile([KJ, HFREE], f32)
            rd_b0 = const_pool.tile([KJ, HFREE], f32)
            rd_b1 = const_pool.tile([KJ, HFREE], f32)
            rds = [[rd_a0, rd_a1], [rd_b0, rd_b1]]
            yhat0 = const_pool.tile([OUT, BS], f32)
            yhat1 = const_pool.tile([OUT, BS], f32)
            yhats = [yhat0, yhat1]
            # group symbol buffer: must persist across YGRP steps
            vgrp = const_pool.tile([OUT, YGRP * BS], i8)

            # din compander decode constants: x = c1*q + c3*q^3, q int8
            amax_c1 = const_pool.tile([KDATA, 1], f32)
            amax_c3 = const_pool.tile([KDATA, 1], f32)
            for src, off, n in wall(WALL_C1, WALL_C1 + 1, 0, KDATA):
                nc.sync.dma_start(amax_c1[:, off : off + n], src)
            for src, off, n in wall(WALL_C3, WALL_C3 + 1, 0, KDATA):
                nc.sync.dma_start(amax_c3[:, off : off + n], src)

            def stage_data(t, h, rd_tile):
                # d(t,h) int8 -> companded f32 -> rd rows 100:121 slice 0.
                # (compute engines can't write at partition 100; DMA can.)
                s8 = tmp_pool.tile([KDATA, HB], i8, tag=f"s8{h}")
                u1 = tmp_pool.tile([KDATA, HB], f32, tag=f"u1{h}")
                u2 = tmp_pool.tile([KDATA, HB], f32, tag=f"u2{h}")
                u3 = tmp_pool.tile([KDATA, HB], f32, tag=f"u3{h}")
                xx = tmp_pool.tile([KDATA, HB], f32, tag=f"xx{h}")
                nc.sync.dma_start(s8[:], dslice(t, h))
                nc.scalar.copy(u1[:], s8[:])
                if D_CUBIC_W > 0:
                    nc.scalar.square(u2[:], u1[:])
                    # p = u2*c3 + c1 ; x = u1*p
                    nc.vector.tensor_scalar(
                        u3[:], u2[:], amax_c3[:], amax_c1[:],
                        mybir.AluOpType.mult, mybir.AluOpType.add,
                    )
                    nc.vector.tensor_tensor(
                        xx[:], u1[:], u3[:], op=mybir.AluOpType.mult
                    )
                else:
                    nc.vector.tensor_scalar(
                        xx[:], u1[:], amax_c1[:], None, mybir.AluOpType.mult
                    )
                nc.sync.dma_start(rd_tile[NPM : NPM + KDATA, 0:HB], xx[:])

            for h in range(NH):
                for p in range(2):
                    nc.vector.memset(rds[h][p][:], 0.0)
                    for src, off, n in wall(WALL_ONES, WALL_ONES + HB, 0, 1):
                        nc.sync.dma_start(
                            rds[h][p][KJ - 1 : KJ, off : off + n], src
                        )

            rep_ctx = (
                tc.For_i(0, n_repeat, 1)
                if n_repeat > 1
                else contextlib.nullcontext()
            )
            with rep_ctx:
                for h in range(NH):
                    for m in range(NMOD):
                        nc.vector.tensor_scalar_add(
                            pres[h][0][:, m * HB : (m + 1) * HB],
                            zf32[:], x0m[:, m : m + 1],
                        )
                    th0 = tmp_pool.tile([NPM, HFREE], f32, tag=f"init{h}")
                    nc.scalar.activation(
                        th0[:], pres[h][0][:],
                        mybir.ActivationFunctionType.Tanh,
                    )
                    nc.vector.tensor_tensor(
                        rds[h][0][0:NPM, :], th0[:], zeros[:],
                        op=mybir.AluOpType.max,
                    )
                    stage_data(0, h, rds[h][0])
                    if n_steps > 1:
                        stage_data(1, h, rds[h][1])
                for p in range(2):
                    nc.vector.memset(yhats[p][:], 0.0)

                ybuf = None
                scbuf = None

                def encode_y(s, ytmp):
                    """DPCM-encode step s's y (both halves) into vgrp/scbuf;
                    pack group when it completes."""
                    g = s % YGRP
                    dt_ = tmp_pool.tile([OUT, BS], f32, tag="dpd")
                    m = tmp_pool.tile([OUT, 1], f32, tag="dpm")
                    scb = tmp_pool.tile([OUT, 1], bf16, tag="dpsb")
                    scf = tmp_pool.tile([OUT, 1], f32, tag="dpsf")
                    rinv = tmp_pool.tile([OUT, 1], f32, tag="dpri")
                    vf = tmp_pool.tile([OUT, BS], f32, tag="dpvf")
                    qf = tmp_pool.tile([OUT, BS], f32, tag="dpqf")
                    qs = tmp_pool.tile([OUT, BS], f32, tag="dpqs")
                    yh_c = yhats[s % 2]
                    yh_n = yhats[(s + 1) % 2]

                    nc.vector.tensor_tensor(
                        dt_[:], ytmp[:], yh_c[:], op=mybir.AluOpType.subtract
                    )
                    nc.vector.tensor_reduce(
                        m[:], dt_[:], mybir.AxisListType.X,
                        mybir.AluOpType.max, apply_absolute_value=True,
                    )
                    # s = max(m/qm, tiny)  (bf16 on the wire AND in the loop)
                    nc.vector.tensor_scalar(
                        scb[:], m[:], 1.0 / QM_EFF, 1e-30,
                        mybir.AluOpType.mult, mybir.AluOpType.max,
                    )
                    nc.scalar.copy(scf[:], scb[:])
                    nc.scalar.copy(scbuf[:, s % ch : s % ch + 1], scb[:])
                    nc.vector.reciprocal(rinv[:], scf[:])
                    # v = d*rinv + Q_OFF  in [0, 2*Q_OFF]
                    nc.vector.tensor_scalar(
                        vf[:], dt_[:], rinv[:], Q_OFF,
                        mybir.AluOpType.mult, mybir.AluOpType.add,
                    )
                    # round via f32->int8 convert (saturates at 127, fine)
                    nc.scalar.copy(vgrp[:, g * BS : (g + 1) * BS], vf[:])
                    # qf = v - Q_OFF back in f32; yhat' = yhat + qf*s
                    nc.scalar.activation(
                        qf[:], vgrp[:, g * BS : (g + 1) * BS],
                        mybir.ActivationFunctionType.Copy, bias=-Q_OFF,
                    )
                    nc.vector.tensor_scalar_mul(qs[:], qf[:], scf[:])
                    nc.vector.tensor_tensor(
                        yh_n[:], yh_c[:], qs[:], op=mybir.AluOpType.add
                    )

                    if g == YGRP - 1 or s == n_steps - 1:
                        _pack_group(s // YGRP, g + 1)

                def _pack_group(gi, nv):
                    """pack nv (<=YGRP) int8 symbols from vgrp -> YGB byte
                    planes in ybuf. SSA-style scratch (no op reads+writes
                    the same tile)."""
                    i32t = [
                        tmp_pool.tile(
                            [OUT, BS], i32, tag=f"pk{k}", name=f"pk{k}"
                        )
                        for k in range(YGRP)
                    ]
                    for k in range(YGRP):
                        if k < nv:
                            nc.scalar.copy(
                                i32t[k][:], vgrp[:, k * BS : (k + 1) * BS]
                            )
                        else:
                            nc.vector.memset(i32t[k][:], 0)
                    ta_t = tmp_pool.tile([OUT, BS], i32, tag="pa0")
                    tb_t = tmp_pool.tile([OUT, BS], i32, tag="pa1")
                    tc_t = tmp_pool.tile([OUT, BS], i32, tag="pa2")
                    td_t = tmp_pool.tile([OUT, BS], i32, tag="pa3")
                    bts = tmp_pool.tile([OUT, YGB * BS], i32, tag="pb")
                    ta, tb, tc_, td = ta_t[:], tb_t[:], tc_t[:], td_t[:]

                    def sh(dst, src, k):
                        op = (
                            mybir.AluOpType.logical_shift_left
                            if k >= 0
                            else mybir.AluOpType.arith_shift_right
                        )
                        nc.vector.tensor_scalar(dst, src, abs(k), None, op)

                    def orr(dst, a, b):
                        nc.vector.tensor_tensor(
                            dst, a, b, op=mybir.AluOpType.bitwise_or
                        )

                    def andsh(dst, src, mask, k):
                        # dst = (src & mask) << k, one fused DVE op
                        nc.vector.tensor_scalar(
                            dst, src, mask, k,
                            mybir.AluOpType.bitwise_and,
                            mybir.AluOpType.logical_shift_left,
                        )

                    def emit(plane, a, b):
                        # bts[plane] = (a | b) - 128
                        orr(td, a, b)
                        nc.vector.tensor_scalar(
                            bts[:, plane * BS : (plane + 1) * BS],
                            td, -128, None, mybir.AluOpType.add,
                        )

                    v = [t[:] for t in i32t]
                    if Y_BITS == 6:
                        # b0 = v0<<2 | v1>>4 ; b1 = (v1&15)<<4 | v2>>2
                        # b2 = (v2&3)<<6 | v3
                        sh(ta, v[0], 2); sh(tb, v[1], -4); emit(0, ta, tb)
                        andsh(ta, v[1], 15, 4); sh(tb, v[2], -2)
                        emit(1, ta, tb)
                        andsh(ta, v[2], 3, 6); emit(2, ta, v[3])
                    else:
                        # b0=v0<<3|v1>>2; b1=(v1&3)<<6|v2<<1|v3>>4
                        # b2=(v3&15)<<4|v4>>1; b3=(v4&1)<<7|v5<<2|v6>>3
                        # b4=(v6&7)<<5|v7
                        sh(ta, v[0], 3); sh(tb, v[1], -2); emit(0, ta, tb)
                        andsh(ta, v[1], 3, 6); sh(tb, v[2], 1)
                        orr(tc_, ta, tb); sh(tb, v[3], -4); emit(1, tc_, tb)
                        andsh(ta, v[3], 15, 4); sh(tb, v[4], -1)
                        emit(2, ta, tb)
                        andsh(ta, v[4], 1, 7); sh(tb, v[5], 2)
                        orr(tc_, ta, tb); sh(tb, v[6], -3); emit(3, tc_, tb)
                        andsh(ta, v[6], 7, 5); emit(4, ta, v[7])
                    gofs = (gi % (ch // YGRP)) * YGB * BS
                    nc.scalar.copy(
                        ybuf[:, gofs : gofs + YGB * BS], bts[:]
                    )

                for t in range(n_steps):
                    s = t - 1          # step whose y this group computes
                    if s % ch == 0 or t == 0:
                        ybuf = yout_pool.tile(
                            [OUT, (ch // YGRP) * YGB * BS], i8, tag="ybuf"
                        )
                        scbuf = yout_pool.tile([OUT, ch], bf16, tag="scbuf")
                    ytmp = tmp_pool.tile([OUT, BS], f32, tag="ytmp")
                    for h in range(NH):
                        pre_cur = pres[h][t % 2]
                        pre_nxt = pres[h][(t + 1) % 2]
                        rd = rds[h][t % 2]
                        rd_nxt = rds[h][(t + 1) % 2]

                        ps = ps_pool.tile([128, 512], f32, tag=f"ps{h}")
                        for k in range(NMOD):
                            rk = rd[0:KJ, k * HB : (k + 1) * HB]
                            for m in range(NMOD):
                                nc.tensor.matmul(
                                    ps[0:NPM, m * HB : (m + 1) * HB],
                                    jt[:, (k * NMOD + m) * NPM : (k * NMOD + m) * NPM + NPM],
                                    rk,
                                    start=(k == 0 and m == 0),
                                    stop=False,
                                )
                        nc.tensor.matmul(
                            ps[0:OUT, HFREE : HFREE + HB],
                            fct[:],
                            rd[0:KJ, 0:HB],
                            start=False,
                            stop=True,
                        )
                        # Euler step on DVE: pre' = 0.9*pre + psum
                        th = tmp_pool.tile([NPM, HFREE], f32, tag=f"th{h}")
                        sc = tmp_pool.tile([NPM, HFREE], f32, tag=f"sc{h}")
                        nc.vector.tensor_scalar_mul(sc[:], pre_cur[:], 0.9)
                        nc.vector.tensor_tensor(
                            pre_nxt[:], sc[:], ps[0:NPM, 0:HFREE],
                            op=mybir.AluOpType.add,
                        )
                        nc.scalar.activation(
                            th[:], pre_nxt[:],
                            mybir.ActivationFunctionType.Tanh,
                        )
                        nc.vector.tensor_tensor(
                            rd_nxt[0:NPM, :], th[:], zeros[:],
                            op=mybir.AluOpType.max,
                        )
                        if t > 0:
                            # y_{t-1} of this half -> ytmp (+bias)
                            nc.vector.tensor_scalar_add(
                                ytmp[:, h * HB : (h + 1) * HB],
                                ps[0:OUT, HFREE : HFREE + HB],
                                fcb[:],
                            )
                        if t + 2 < n_steps:
                            stage_data(t + 2, h, rd)
                    if t > 0:
                        encode_y(s, ytmp)
                        if s % ch == ch - 1:
                            c0 = s - ch + 1
                            nc.sync.dma_start(
                                y_ap[:, (c0 // YGRP) * YGB * BS
                                     : (c0 // YGRP) * YGB * BS
                                     + (ch // YGRP) * YGB * BS],
                                ybuf[:],
                            )
                            nc.sync.dma_start(sc_ap[:, c0 : c0 + ch], scbuf[:])

                # trailing: y of the last step, per half
                s = n_steps - 1
                if s % ch == 0:
                    ybuf = yout_pool.tile(
                        [OUT, (ch // YGRP) * YGB * BS], i8, tag="ybuf"
                    )
                    scbuf = yout_pool.tile([OUT, ch], bf16, tag="scbuf")
                ytmp = tmp_pool.tile([OUT, BS], f32, tag="ytmp")
                for h in range(NH):
                    ps = ps_pool.tile([128, 512], f32, tag=f"ps{h}")
                    nc.tensor.matmul(
                        ps[0:OUT, HFREE : HFREE + HB],
                        fct[:],
                        rds[h][n_steps % 2][0:KJ, 0:HB],
                        start=True,
                        stop=True,
                    )
                    nc.vector.tensor_scalar_add(
                        ytmp[:, h * HB : (h + 1) * HB],
                        ps[0:OUT, HFREE : HFREE + HB],
                        fcb[:],
                    )
                encode_y(s, ytmp)
                c0 = s - s % ch
                ng0 = c0 // YGRP
                nglast = _ygroups(n_steps) - ng0
                nc.sync.dma_start(
                    y_ap[:, ng0 * YGB * BS : (ng0 + nglast) * YGB * BS],
                    ybuf[:, : nglast * YGB * BS],
                )
                nc.sync.dma_start(
                    sc_ap[:, c0 : n_steps], scbuf[:, : n_steps - c0]
                )

    nc.compile()
    return nc


def _get_program(n_steps: int, n_repeat: int = 1):
    key = (n_steps, Y_BITS, D_CUBIC_W, W_MODE, n_repeat)
    if key not in _BUILD_CACHE:
        _BUILD_CACHE[key] = _build_program(n_steps, n_repeat)
    return _BUILD_CACHE[key]


def _plan_up_chunks(n_steps: int):
    k = min(N_UP_CHUNKS, n_steps)
    base = n_steps // k
    rem = n_steps - base * k
    return [base + (1 if i < rem else 0) for i in range(k)]


def _make_encode_lut(amax: float):
    """uint16-indexed LUT: i = rint(x*32767/amax)+32767 -> int8 level index.

    Built by exact inversion on the 65535 bin centers (cheap), so the
    per-element encode is one mul+rint+gather.
    """
    w = D_CUBIC_W
    # index j corresponds to i = j - 32768 (j = int16(i) ^ 0x8000)
    xs = ((np.arange(65536, dtype=np.float64) - 32768.0) / 32767.0) * amax
    if w <= 0:
        u = xs / amax
    else:
        # solve w*u + (1-w)*u^3 = x/amax by Cardano (monotone odd cubic)
        a = 1.0 - w
        p = w / a
        qq = -np.abs(xs) / (amax * a)
        disc = np.sqrt((qq * qq) / 4.0 + (p ** 3) / 27.0)
        u = np.cbrt(-qq / 2.0 + disc) + np.cbrt(-qq / 2.0 - disc)
        u = np.copysign(u, xs)
    q = np.rint(np.clip(u, -1.0, 1.0) * 127.0).astype(np.int8)
    return q


def _cubic_encode(data_f: np.ndarray, amax: float, lut):
    i = np.rint(data_f * np.float32(32767.0 / amax)).astype(np.int16)
    return lut[i.view(np.uint16).ravel() ^ 0x8000].reshape(data_f.shape)


def _prep_weights(J, I, S, Bb, x0, fc_w, fc_b, amax):
    f32 = np.float32
    dsc = 1.0  # data scale rides in the compander constants, not weights
    Jp = 0.1 * np.asarray(J, f32)
    Ip = 0.1 * np.asarray(I, f32)
    Sp = 0.1 * np.asarray(S, f32)
    Bbp = 0.1 * np.asarray(Bb, f32)

    jt = np.zeros((KJ, 9, NPM), f32)
    for k in range(NMOD):
        for m in range(NMOD):
            blk = Jp[m * NPM : (m + 1) * NPM, k * NPM : (k + 1) * NPM]
            jt[:NPM, k * NMOD + m, :NPM] = blk.T
            if k == 0:
                jt[NPM : NPM + NF, k * NMOD + m, :NPM] = (
                    Ip[m * NPM : (m + 1) * NPM, :].T
                )
                jt[NPM + NF, k * NMOD + m, :NPM] = Sp[m * NPM : (m + 1) * NPM, 0]
                jt[NPM + NF + 1, k * NMOD + m, :NPM] = (
                    Bbp[m * NPM : (m + 1) * NPM, 0]
                )

    wall = np.zeros((KJ, WALL_COLS), f32)
    wall[:, : 9 * NPM] = jt.reshape(KJ, 9 * NPM)
    wall[:NPM, WALL_FCT : WALL_FCT + OUT] = np.asarray(fc_w, f32).T
    wall[0, WALL_ONES : WALL_ONES + HB] = 1.0
    # compander decode constants per data row: x = c1*q + c3*q^3
    w = D_CUBIC_W
    if w > 0:
        c1 = amax * w / 127.0
        c3 = amax * (1.0 - w) / (127.0 ** 3)
    else:
        c1 = amax / 127.0
        c3 = 0.0
    wall[:KDATA, WALL_ONES + HB] = c1
    wall[:KDATA, WALL_ONES + HB + 1] = c3
    wall[:OUT, WALL_FCB] = np.asarray(fc_b, f32)
    wall[:NPM, WALL_X0 : WALL_X0 + NMOD] = (
        np.asarray(x0, f32).reshape(NMOD, NPM).T
    )
    if W_MODE == "gather":
        # core c uploads column block c: global [8*KJ, WBLK]
        return np.ascontiguousarray(
            wall.reshape(KJ, N_CORES, WBLK).transpose(1, 0, 2)
        ).reshape(N_CORES * KJ, WBLK)
    return np.ascontiguousarray(
        np.broadcast_to(wall, (N_CORES, KJ, WALL_COLS))
    ).reshape(N_CORES * KJ, WALL_COLS)


def _prep_din_chunk(dat_f, t0, c):
    """[T,21,B] f32 -> companded int8 [8*21, c*BS], core-major."""
    amax = _prep_din_chunk.amax
    q = _cubic_encode(dat_f[t0 : t0 + c], amax, _prep_din_chunk.lut)
    return np.ascontiguousarray(
        q.reshape(c, KDATA, N_CORES, BS).transpose(2, 1, 0, 3)
    ).reshape(N_CORES * KDATA, c * BS)


class _Runner:
    """Persistent jitted shard_map callable for one compiled program."""

    def __init__(self, nc):
        import jax
        import jax.numpy as jnp
        from jax.sharding import Mesh, PartitionSpec
        from jax.experimental.shard_map import shard_map
        from concourse.bass2jax import (
            _bass_exec_p,
            install_neuronx_cc_hook,
            partition_id_tensor,
        )

        install_neuronx_cc_hook()
        self.nc = nc
        partition_name = (
            nc.partition_id_tensor.name if nc.partition_id_tensor else None
        )

        in_names, out_names, out_avals, zero_shapes = [], [], [], []
        for alloc in nc.m.functions[0].allocations:
            if not isinstance(alloc, mybir.MemoryLocationSet):
                continue
            name = alloc.memorylocations[0].name
            if alloc.kind == "ExternalInput":
                if name != partition_name:
                    in_names.append(name)
            elif alloc.kind == "ExternalOutput":
                np_dt = mybir.dt.np(alloc.dtype)
                out_avals.append(
                    jax.core.ShapedArray(tuple(alloc.tensor_shape), np_dt)
                )
                out_names.append(name)
                zero_shapes.append((tuple(alloc.tensor_shape), np_dt))
        assert out_names[0] == "y", out_names
        self.in_names = in_names
        self.out_names = out_names

        n_params = len(in_names)
        n_outs = len(out_names)
        all_in_names = list(in_names) + list(out_names)
        if partition_name is not None:
            all_in_names.append(partition_name)

        def _body(*args):
            operands = list(args)
            if partition_name is not None:
                operands.append(partition_id_tensor())
            outs = _bass_exec_p.bind(
                *operands,
                out_avals=tuple(out_avals),
                in_names=tuple(all_in_names),
                out_names=tuple(out_names),
                lowering_input_output_aliases=(),
                sim_require_finite=True,
                sim_require_nnan=True,
                nc=nc,
            )
            return tuple(outs)

        devices = jax.devices()[:N_CORES]
        mesh = Mesh(np.asarray(devices), ("core",))
        in_specs = (PartitionSpec("core"),) * (n_params + n_outs)
        out_specs = (PartitionSpec("core"),) * n_outs
        self.mesh = mesh
        self.spec = jax.sharding.NamedSharding(mesh, PartitionSpec("core"))
        self.sharded = jax.jit(
            shard_map(
                _body, mesh=mesh, in_specs=in_specs, out_specs=out_specs,
                check_rep=False,
            ),
            keep_unused=True,
        )
        self.zeros = [
            jnp.zeros((N_CORES * shp[0], *shp[1:]), dt)
            for shp, dt in zero_shapes
        ]

    def __call__(self, *ins):
        return self.sharded(*ins, *self.zeros)


def _get_runner(n_steps: int, n_repeat: int = 1):
    key = (n_steps, Y_BITS, D_CUBIC_W, W_MODE, n_repeat)
    if key not in _RUNNER_CACHE:
        _RUNNER_CACHE[key] = _Runner(_get_program(n_steps, n_repeat))
    return _RUNNER_CACHE[key]


def _decode_shard(final, ybytes, scales, c, n_steps):
    """6/5-bit DPCM shard -> final[:, c*BS:(c+1)*BS, :] f32."""
    ng = _ygroups(n_steps)
    # device wrote (byte - 128) as int8; uint8 view of that is byte XOR 128
    bb = (ybytes.view(np.uint8) ^ 128).astype(np.int16)
    bb = bb.reshape(OUT, ng, YGB, BS)
    v = np.empty((OUT, ng, YGRP, BS), np.int16)
    b = [bb[:, :, k] for k in range(YGB)]
    if Y_BITS == 6:
        v[:, :, 0] = b[0] >> 2
        v[:, :, 1] = ((b[0] & 3) << 4) | (b[1] >> 4)
        v[:, :, 2] = ((b[1] & 15) << 2) | (b[2] >> 6)
        v[:, :, 3] = b[2] & 63
    else:
        v[:, :, 0] = b[0] >> 3
        v[:, :, 1] = ((b[0] & 7) << 2) | (b[1] >> 6)
        v[:, :, 2] = (b[1] >> 1) & 31
        v[:, :, 3] = ((b[1] & 1) << 4) | (b[2] >> 4)
        v[:, :, 4] = ((b[2] & 15) << 1) | (b[3] >> 7)
        v[:, :, 5] = (b[3] >> 2) & 31
        v[:, :, 6] = ((b[3] & 3) << 3) | (b[4] >> 5)
        v[:, :, 7] = b[4] & 31
    v = v.reshape(OUT, ng * YGRP, BS)[:, :n_steps]
    # y_t = cumsum_t(q*s) = cumsum(v*s) - Q_OFF*cumsum(s)
    s = scales.astype(np.float32)[:, :, None]        # [OUT, T, 1]
    ys = v.astype(np.float32)
    ys *= s
    np.cumsum(ys, axis=1, out=ys)
    cs = np.cumsum(s, axis=1, dtype=np.float32)
    ys -= Q_OFF * cs
    final[:, c * BS : (c + 1) * BS, :] = ys.transpose(1, 2, 0)


def run_sharded(inputs: dict, n_steps: int = T):
    """Compile (cached), run on 8 cores, return [T, B, OUT] f32."""
    import jax
    import os
    import time as _time
    from concurrent.futures import ThreadPoolExecutor

    dbg = os.environ.get("K_DEBUG_TIMING")
    tt0 = _time.time()

    def mark(lbl):
        if dbg:
            print(f"  [{(_time.time()-tt0)*1e3:7.1f}ms] {lbl}", flush=True)

    runner = _get_runner(n_steps)
    spec = runner.spec

    data = np.asarray(inputs["data"], np.float32)[:n_steps]
    amax = max(float(data.max()), -float(data.min()), 1e-30)
    _prep_din_chunk.amax = amax
    _prep_din_chunk.lut = _make_encode_lut(amax)
    mark("amax+lut")

    # upload pipeline: a single uploader thread keeps the wire busy in
    # submission order while the main thread preps the next chunk.
    up_ex = ThreadPoolExecutor(1)

    def put(arr):
        x = jax.device_put(arr, spec)
        x.block_until_ready()
        return x

    wglob = _prep_weights(
        inputs["J"], inputs["I"], inputs["S"], inputs["Bb"],
        inputs["x0"], inputs["fc_w"], inputs["fc_b"], amax,
    )
    futs = {"w": up_ex.submit(put, wglob)}
    mark("w prepped+queued")
    chunks = _plan_up_chunks(n_steps)
    t0 = 0
    for i, c in enumerate(chunks):
        arr = _prep_din_chunk(data, t0, c)
        futs[f"din{i}"] = up_ex.submit(put, arr)
        mark(f"din{i} prepped+queued")
        t0 += c
    dev_in = {k: f.result() for k, f in futs.items()}
    up_ex.shutdown(wait=False)
    mark("uploads done")

    outs = runner(*[dev_in[n] for n in runner.in_names])
    y_g, sc_g = outs[0], outs[1]

    final = np.empty((n_steps, B, OUT), np.float32)
    ysh = sorted(y_g.addressable_shards, key=lambda s: s.index[0].start)
    ssh = sorted(sc_g.addressable_shards, key=lambda s: s.index[0].start)
    # staged fetch: keep ~2 y shards in flight so the wire stays busy
    # while the main thread decodes the shard that just landed (decode
    # ~26ms < ~55ms shard wire time, so decode fully hides).
    INFLIGHT = 2
    for c in range(min(INFLIGHT, N_CORES)):
        ysh[c].data.copy_to_host_async()
        ssh[c].data.copy_to_host_async()
    mark("exec dispatched, first fetches queued")
    for c in range(N_CORES):
        yb = np.asarray(ysh[c].data)
        scb = np.asarray(ssh[c].data)
        if c + INFLIGHT < N_CORES:
            ysh[c + INFLIGHT].data.copy_to_host_async()
            ssh[c + INFLIGHT].data.copy_to_host_async()
        mark(f"shard {c} fetched")
        _decode_shard(final, yb, scb, c, n_steps)
    mark("decoded")
    return final


def kernel(data, J, I, S, Bb, x0, fc_w, fc_b):
    return run_sharded(
        dict(data=data, J=J, I=I, S=S, Bb=Bb, x0=x0, fc_w=fc_w, fc_b=fc_b)
    )


# revision 18
# speedup vs baseline: 1.2685x; 1.2685x over previous
"""Trainium2 Bass kernel for the MichaelsRNN forward pass.

Reference math (per time step t, per batch element b):
    recur = r @ J.T
    inp   = image.T @ I.T + hold.T * S.T
    pre   = 0.9*x + 0.1*(recur + inp + Bb.T)     # Euler step dt/tau = 1/10
    out   = retanh(pre) = max(tanh(pre), 0)
    y     = out[:, :100] @ fc_w.T + fc_b
    carry = (pre, out)

Sharding: data-parallel over batch. B=1024 over 8 cores = 128/core, in
two phase-shifted half-batches of 64 (PE runs half B's matmul group
while ACT/DVE run half A's elementwise).

Wall-clock here is dominated by the axon tunnel: ~40 MB/s TOTAL,
shared between directions and across all 8 devices (measured: no
duplex win, no multi-stream win, no D2H compression). Device exec is
~free (<2% of wall). So the kernel is engineered to minimize wire
bytes; precision is the scarce resource (rel-err gate 2e-2, error
amplified by the recurrent J with spectral radius ~1.2):

  - weights: f32 (PE f32 matmul is free here; bf16 weights alone cost
    0.6% rel err). Uploaded as ONE sharded copy (122x128 slice per
    core) and AllGather'd on device (0.5MB instead of 4MB x8).
  - din: int8 with a cubic companding curve (decode on device is a
    2-op poly; finer steps near 0 cut the 0.9% uniform-int8 error to
    ~0.6%). 10.75MB.
  - y: closed-loop DPCM, per-(step,row) scale: the device keeps the
    decoder state yhat, quantizes d = y_t - yhat to Y_BITS symbols
    with scale s = rowmax|d|/qm (shipped bf16), packs symbols into
    bytes (6-bit: 4 vals -> 3B = 19.2MB; 5-bit: 8 vals -> 5B = 16MB),
    vs 25.6MB for direct int8 -- and with LESS quantization error
    (deltas are ~8x smaller than y).
    Host decode = unpack + cumsum(q*s); it need not be bit-exact with
    the device's yhat (f32 drift ~1e-7 << budget).

Host-side (1 CPU core!) work is pipelined under the wire: din is
companded/packed per chunk while the previous chunk uploads; y shards
decode while the next shard downloads.

Per half-step, ONE PSUM accumulation group in one bank:
    9x J matmul      lhsT[122,100]=[0.1J[m,k].T ; k==0?[0.1I;0.1S;0.1Bb]:0]
                     rhs=rd_h[0:122, k]  (k0m0 opens the group)
    1x fc matmul     lhsT=[fc_w.T;0] [122,50] rhs=rd_h = y of t-1 (stop)
The Euler step pre' = 0.9*pre + psum runs on DVE; r = retanh on
ACT(tanh)+DVE(max). y_{t-1} is computed inside step t's group (its
input r_{t-1} is still live), so it costs no extra PSUM group.

State per half (ping-pong on step parity):
    rd_{h,p} [122, 192] f32: rows 0:100 = r; rows 100:121 of module
        slice 0 = the step's companded [image;hold] (staged 2 steps
        ahead); row 121 slice 0 = ones (drives Bb); rows 100:122 of
        slices 1,2 = zeros (meet zero weights only).
"""

import numpy as np
import ml_dtypes

import concourse.bass as bass  # noqa: F401
import concourse.tile as tile
from concourse import bacc, mybir

NPM = 100
NMOD = 3
NN = 300
NF = 20
OUT = 50
T = 500
B = 1024
N_CORES = 8
BS = B // N_CORES      # 128 batch per core
NH = 2                 # phase-shifted half-batches
HB = BS // NH          # 64
HFREE = NMOD * HB      # 192
KDATA = NF + 1         # 21 data rows on the wire (image, hold)
KD = KDATA + 1         # 22 data rows in SBUF (plus ones)
KJ = NPM + KD          # 122

Y_BITS = 5             # 6: pack 4 vals->3B; 5: pack 8 vals->5B
D_CUBIC_W = 0.45       # din compander: dec(u) = amax*(w*u+(1-w)*u^3); 0 -> uniform
W_MODE = "gather"      # "gather": 1 sharded copy + on-device AllGather; "repl": x8
N_UP_CHUNKS = 2        # din upload pipeline granularity (fewer -> less per-
                       # transfer overhead; prep is LUT-fast so 2 suffices)

# y wire geometry
if Y_BITS == 6:
    YGRP, YGB = 4, 3          # 4 steps -> 3 byte-planes
    QM_EFF = 30.5             # |q| <= 31 incl. reciprocal slop; v=q+31 in [0,62]
    Q_OFF = 31.0
    CH = 20                   # steps per y-out chunk (multiple of YGRP)
else:
    YGRP, YGB = 8, 5
    QM_EFF = 14.5
    Q_OFF = 15.0
    CH = 24

# f32 weight pack, per-core shard is a [KJ, 128] column block; everything
# past col 896 lives in core 7's block so gather-mode DMAs stay simple
WALL_JT = 9 * NPM                  # cols 0:900   jt
WALL_FCT = 904                     # cols 904:954 fct
WALL_ONES = 954                    # cols 954:1018  ones row (row 0)
WALL_C1 = 1018                     # col 1018     compander c1 (rows 0:KDATA)
WALL_C3 = 1019                     # col 1019     compander c3 (rows 0:KDATA)
WALL_FCB = 1020                    # col 1020     fcb (rows 0:OUT)
WALL_X0 = 1021                     # cols 1021:1024  x0 modules
WALL_COLS = 1024
WBLK = WALL_COLS // N_CORES        # 128

_BUILD_CACHE: dict = {}
_RUNNER_CACHE: dict = {}


def _ceil_div(a, b):
    return -(-a // b)


def _ygroups(n_steps):
    return _ceil_div(n_steps, YGRP)


def _build_program(n_steps: int, n_repeat: int = 1):
    """Build + compile the Bass program (value-independent)."""
    f32 = mybir.dt.float32
    bf16 = mybir.dt.bfloat16
    i8 = mybir.dt.int8
    i32 = mybir.dt.int32
    import contextlib

    nc = bacc.Bacc(
        "TRN2", target_bir_lowering=False, debug=False, num_devices=N_CORES
    )

    chunks = _plan_up_chunks(n_steps)
    din_aps = []
    for i, c in enumerate(chunks):
        din_aps.append(
            nc.dram_tensor(f"din{i}", [KDATA, c * BS], i8, kind="ExternalInput").ap()
        )
    if W_MODE == "gather":
        w_ap = nc.dram_tensor("w", [KJ, WBLK], f32, kind="ExternalInput").ap()
    else:
        w_ap = nc.dram_tensor("w", [KJ, WALL_COLS], f32, kind="ExternalInput").ap()
    ng = _ygroups(n_steps)
    y_ap = nc.dram_tensor(
        "y", [OUT, ng * YGB * BS], i8, kind="ExternalOutput"
    ).ap()
    sc_ap = nc.dram_tensor(
        "sc", [OUT, n_steps], bf16, kind="ExternalOutput"
    ).ap()

    ch = min(CH, n_steps)
    cum = np.cumsum([0] + chunks)

    def dslice(t, h):
        i = int(np.searchsorted(cum, t, side="right")) - 1
        off = (t - cum[i]) * BS + h * HB
        return din_aps[i][:, off : off + HB]

    with tile.TileContext(nc) as tc:
        with contextlib.ExitStack() as ctx:
            const_pool = ctx.enter_context(tc.tile_pool(name="const", bufs=1))
            yout_pool = ctx.enter_context(tc.tile_pool(name="yout", bufs=2))
            tmp_pool = ctx.enter_context(tc.tile_pool(name="tmp", bufs=2))
            ps_pool = ctx.enter_context(
                tc.tile_pool(name="ps", bufs=2, space="PSUM")
            )

            # ---- weights: AllGather one sharded copy, or use replicated ----
            if W_MODE == "gather":
                dram_pool = ctx.enter_context(
                    tc.tile_pool(name="dramw", bufs=1, space="DRAM")
                )
                win = dram_pool.tile([KJ, WBLK], f32)
                wg = dram_pool.tile([N_CORES * KJ, WBLK], f32)
                nc.gpsimd.dma_start(win[:], w_ap)
                nc.gpsimd.collective_compute(
                    "AllGather",
                    mybir.AluOpType.bypass,
                    replica_groups=[list(range(N_CORES))],
                    ins=[win[:].opt()],
                    outs=[wg[:].opt()],
                )

                def wall(c0, c1, r0=0, r1=KJ):
                    """list of (dram slice, dst col offset) covering cols c0:c1."""
                    outl = []
                    for c in range(N_CORES):
                        a = max(c0, c * WBLK)
                        b = min(c1, (c + 1) * WBLK)
                        if a < b:
                            outl.append(
                                (
                                    wg[c * KJ + r0 : c * KJ + r1,
                                       a - c * WBLK : b - c * WBLK],
                                    a - c0,
                                    b - a,
                                )
                            )
                    return outl
            else:

                def wall(c0, c1, r0=0, r1=KJ):
                    return [(w_ap[r0:r1, c0:c1], 0, c1 - c0)]

            jt = const_pool.tile([KJ, 9 * NPM], f32)
            for src, off, n in wall(0, 9 * NPM):
                nc.sync.dma_start(jt[:, off : off + n], src)
            fct = const_pool.tile([KJ, OUT], f32)
            for src, off, n in wall(WALL_FCT, WALL_FCT + OUT):
                nc.sync.dma_start(fct[:, off : off + n], src)
            fcb = const_pool.tile([OUT, 1], f32)
            for src, off, n in wall(WALL_FCB, WALL_FCB + 1, 0, OUT):
                nc.sync.dma_start(fcb[:, off : off + n], src)
            x0m = const_pool.tile([NPM, NMOD], f32)
            for src, off, n in wall(WALL_X0, WALL_X0 + NMOD, 0, NPM):
                nc.sync.dma_start(x0m[:, off : off + n], src)

            zeros = const_pool.tile([NPM, HFREE], f32)
            nc.vector.memset(zeros[:], 0.0)
            zf32 = const_pool.tile([NPM, HB], f32)
            nc.vector.memset(zf32[:], 0.0)

            pre_a0 = const_pool.tile([NPM, HFREE], f32)
            pre_a1 = const_pool.tile([NPM, HFREE], f32)
            pre_b0 = const_pool.tile([NPM, HFREE], f32)
            pre_b1 = const_pool.tile([NPM, HFREE], f32)
            pres = [[pre_a0, pre_a1], [pre_b0, pre_b1]]
            rd_a0 = const_pool.tile([KJ, HFREE], f32)
            rd_a1 = const_pool.tile([KJ, HFREE], f32)
            rd_b0 = const_pool.tile([KJ, HFREE], f32)
            rd_b1 = const_pool.tile([KJ, HFREE], f32)
            rds = [[rd_a0, rd_a1], [rd_b0, rd_b1]]
            yhat0 = const_pool.tile([OUT, BS], f32)
            yhat1 = const_pool.tile([OUT, BS], f32)
            yhats = [yhat0, yhat1]
            # group symbol buffer: must persist across YGRP steps
            vgrp = const_pool.tile([OUT, YGRP * BS], i8)

            # din compander decode constants: x = c1*q + c3*q^3, q int8
            amax_c1 = const_pool.tile([KDATA, 1], f32)
            amax_c3 = const_pool.tile([KDATA, 1], f32)
            for src, off, n in wall(WALL_C1, WALL_C1 + 1, 0, KDATA):
                nc.sync.dma_start(amax_c1[:, off : off + n], src)
            for src, off, n in wall(WALL_C3, WALL_C3 + 1, 0, KDATA):
                nc.sync.dma_start(amax_c3[:, off : off + n], src)

            def stage_data(t, h, rd_tile):
                # d(t,h) int8 -> companded f32 -> rd rows 100:121 slice 0.
                # (compute engines can't write at partition 100; DMA can.)
                s8 = tmp_pool.tile([KDATA, HB], i8, tag=f"s8{h}")
                u1 = tmp_pool.tile([KDATA, HB], f32, tag=f"u1{h}")
                u2 = tmp_pool.tile([KDATA, HB], f32, tag=f"u2{h}")
                u3 = tmp_pool.tile([KDATA, HB], f32, tag=f"u3{h}")
                xx = tmp_pool.tile([KDATA, HB], f32, tag=f"xx{h}")
                nc.sync.dma_start(s8[:], dslice(t, h))
                nc.scalar.copy(u1[:], s8[:])
                if D_CUBIC_W > 0:
                    nc.scalar.square(u2[:], u1[:])
                    # p = u2*c3 + c1 ; x = u1*p
                    nc.vector.tensor_scalar(
                        u3[:], u2[:], amax_c3[:], amax_c1[:],
                        mybir.AluOpType.mult, mybir.AluOpType.add,
                    )
                    nc.vector.tensor_tensor(
                        xx[:], u1[:], u3[:], op=mybir.AluOpType.mult
                    )
                else:
                    nc.vector.tensor_scalar(
                        xx[:], u1[:], amax_c1[:], None, mybir.AluOpType.mult
                    )
                nc.sync.dma_start(rd_tile[NPM : NPM + KDATA, 0:HB], xx[:])

            for h in range(NH):
                for p in range(2):
                    nc.vector.memset(rds[h][p][:], 0.0)
                    for src, off, n in wall(WALL_ONES, WALL_ONES + HB, 0, 1):
                        nc.sync.dma_start(
                            rds[h][p][KJ - 1 : KJ, off : off + n], src
                        )

            rep_ctx = (
                tc.For_i(0, n_repeat, 1)
                if n_repeat > 1
                else contextlib.nullcontext()
            )
            with rep_ctx:
                for h in range(NH):
                    for m in range(NMOD):
                        nc.vector.tensor_scalar_add(
                            pres[h][0][:, m * HB : (m + 1) * HB],
                            zf32[:], x0m[:, m : m + 1],
                        )
                    th0 = tmp_pool.tile([NPM, HFREE], f32, tag=f"init{h}")
                    nc.scalar.activation(
                        th0[:], pres[h][0][:],
                        mybir.ActivationFunctionType.Tanh,
                    )
                    nc.vector.tensor_tensor(
                        rds[h][0][0:NPM, :], th0[:], zeros[:],
                        op=mybir.AluOpType.max,
                    )
                    stage_data(0, h, rds[h][0])
                    if n_steps > 1:
                        stage_data(1, h, rds[h][1])
                for p in range(2):
                    nc.vector.memset(yhats[p][:], 0.0)

                ybuf = None
                scbuf = None

                def encode_y(s, ytmp):
                    """DPCM-encode step s's y (both halves) into vgrp/scbuf;
                    pack group when it completes."""
                    g = s % YGRP
                    dt_ = tmp_pool.tile([OUT, BS], f32, tag="dpd")
                    m = tmp_pool.tile([OUT, 1], f32, tag="dpm")
                    scb = tmp_pool.tile([OUT, 1], bf16, tag="dpsb")
                    scf = tmp_pool.tile([OUT, 1], f32, tag="dpsf")
                    rinv = tmp_pool.tile([OUT, 1], f32, tag="dpri")
                    vf = tmp_pool.tile([OUT, BS], f32, tag="dpvf")
                    qf = tmp_pool.tile([OUT, BS], f32, tag="dpqf")
                    qs = tmp_pool.tile([OUT, BS], f32, tag="dpqs")
                    yh_c = yhats[s % 2]
                    yh_n = yhats[(s + 1) % 2]

                    nc.vector.tensor_tensor(
                        dt_[:], ytmp[:], yh_c[:], op=mybir.AluOpType.subtract
                    )
                    nc.vector.tensor_reduce(
                        m[:], dt_[:], mybir.AxisListType.X,
                        mybir.AluOpType.max, apply_absolute_value=True,
                    )
                    # s = max(m/qm, tiny)  (bf16 on the wire AND in the loop)
                    nc.vector.tensor_scalar(
                        scb[:], m[:], 1.0 / QM_EFF, 1e-30,
                        mybir.AluOpType.mult, mybir.AluOpType.max,
                    )
                    nc.scalar.copy(scf[:], scb[:])
                    nc.scalar.copy(scbuf[:, s % ch : s % ch + 1], scb[:])
                    nc.vector.reciprocal(rinv[:], scf[:])
                    # v = d*rinv + Q_OFF  in [0, 2*Q_OFF]
                    nc.vector.tensor_scalar(
                        vf[:], dt_[:], rinv[:], Q_OFF,
                        mybir.AluOpType.mult, mybir.AluOpType.add,
                    )
                    # round via f32->int8 convert (saturates at 127, fine)
                    nc.scalar.copy(vgrp[:, g * BS : (g + 1) * BS], vf[:])
                    # qf = v - Q_OFF back in f32; yhat' = yhat + qf*s
                    nc.scalar.activation(
                        qf[:], vgrp[:, g * BS : (g + 1) * BS],
                        mybir.ActivationFunctionType.Copy, bias=-Q_OFF,
                    )
                    nc.vector.tensor_scalar_mul(qs[:], qf[:], scf[:])
                    nc.vector.tensor_tensor(
                        yh_n[:], yh_c[:], qs[:], op=mybir.AluOpType.add
                    )

                    if g == YGRP - 1 or s == n_steps - 1:
                        _pack_group(s // YGRP, g + 1)

                def _pack_group(gi, nv):
                    """pack nv (<=YGRP) int8 symbols from vgrp -> YGB byte
                    planes in ybuf. SSA-style scratch (no op reads+writes
                    the same tile)."""
                    i32t = [
                        tmp_pool.tile(
                            [OUT, BS], i32, tag=f"pk{k}", name=f"pk{k}"
                        )
                        for k in range(YGRP)
                    ]
                    for k in range(YGRP):
                        if k < nv:
                            nc.scalar.copy(
                                i32t[k][:], vgrp[:, k * BS : (k + 1) * BS]
                            )
                        else:
                            nc.vector.memset(i32t[k][:], 0)
                    ta_t = tmp_pool.tile([OUT, BS], i32, tag="pa0")
                    tb_t = tmp_pool.tile([OUT, BS], i32, tag="pa1")
                    tc_t = tmp_pool.tile([OUT, BS], i32, tag="pa2")
                    td_t = tmp_pool.tile([OUT, BS], i32, tag="pa3")
                    bts = tmp_pool.tile([OUT, YGB * BS], i32, tag="pb")
                    ta, tb, tc_, td = ta_t[:], tb_t[:], tc_t[:], td_t[:]

                    def sh(dst, src, k):
                        op = (
                            mybir.AluOpType.logical_shift_left
                            if k >= 0
                            else mybir.AluOpType.arith_shift_right
                        )
                        nc.vector.tensor_scalar(dst, src, abs(k), None, op)

                    def orr(dst, a, b):
                        nc.vector.tensor_tensor(
                            dst, a, b, op=mybir.AluOpType.bitwise_or
                        )

                    def andsh(dst, src, mask, k):
                        # dst = (src & mask) << k, one fused DVE op
                        nc.vector.tensor_scalar(
                            dst, src, mask, k,
                            mybir.AluOpType.bitwise_and,
                            mybir.AluOpType.logical_shift_left,
                        )

                    def emit(plane, a, b):
                        # bts[plane] = (a | b) - 128
                        orr(td, a, b)
                        nc.vector.tensor_scalar(
                            bts[:, plane * BS : (plane + 1) * BS],
                            td, -128, None, mybir.AluOpType.add,
                        )

                    v = [t[:] for t in i32t]
                    if Y_BITS == 6:
                        # b0 = v0<<2 | v1>>4 ; b1 = (v1&15)<<4 | v2>>2
                        # b2 = (v2&3)<<6 | v3
                        sh(ta, v[0], 2); sh(tb, v[1], -4); emit(0, ta, tb)
                        andsh(ta, v[1], 15, 4); sh(tb, v[2], -2)
                        emit(1, ta, tb)
                        andsh(ta, v[2], 3, 6); emit(2, ta, v[3])
                    else:
                        # b0=v0<<3|v1>>2; b1=(v1&3)<<6|v2<<1|v3>>4
                        # b2=(v3&15)<<4|v4>>1; b3=(v4&1)<<7|v5<<2|v6>>3
                        # b4=(v6&7)<<5|v7
                        sh(ta, v[0], 3); sh(tb, v[1], -2); emit(0, ta, tb)
                        andsh(ta, v[1], 3, 6); sh(tb, v[2], 1)
                        orr(tc_, ta, tb); sh(tb, v[3], -4); emit(1, tc_, tb)
                        andsh(ta, v[3], 15, 4); sh(tb, v[4], -1)
                        emit(2, ta, tb)
                        andsh(ta, v[4], 1, 7); sh(tb, v[5], 2)
                        orr(tc_, ta, tb); sh(tb, v[6], -3); emit(3, tc_, tb)
                        andsh(ta, v[6], 7, 5); emit(4, ta, v[7])
                    gofs = (gi % (ch // YGRP)) * YGB * BS
                    nc.scalar.copy(
                        ybuf[:, gofs : gofs + YGB * BS], bts[:]
                    )

                for t in range(n_steps):
                    s = t - 1          # step whose y this group computes
                    if s % ch == 0 or t == 0:
                        ybuf = yout_pool.tile(
                            [OUT, (ch // YGRP) * YGB * BS], i8, tag="ybuf"
                        )
                        scbuf = yout_pool.tile([OUT, ch], bf16, tag="scbuf")
                    ytmp = tmp_pool.tile([OUT, BS], f32, tag="ytmp")
                    for h in range(NH):
                        pre_cur = pres[h][t % 2]
                        pre_nxt = pres[h][(t + 1) % 2]
                        rd = rds[h][t % 2]
                        rd_nxt = rds[h][(t + 1) % 2]

                        ps = ps_pool.tile([128, 512], f32, tag=f"ps{h}")
                        for k in range(NMOD):
                            rk = rd[0:KJ, k * HB : (k + 1) * HB]
                            for m in range(NMOD):
                                nc.tensor.matmul(
                                    ps[0:NPM, m * HB : (m + 1) * HB],
                                    jt[:, (k * NMOD + m) * NPM : (k * NMOD + m) * NPM + NPM],
                                    rk,
                                    start=(k == 0 and m == 0),
                                    stop=False,
                                )
                        nc.tensor.matmul(
                            ps[0:OUT, HFREE : HFREE + HB],
                            fct[:],
                            rd[0:KJ, 0:HB],
                            start=False,
                            stop=True,
                        )
                        # Euler step on DVE: pre' = 0.9*pre + psum
                        th = tmp_pool.tile([NPM, HFREE], f32, tag=f"th{h}")
                        sc = tmp_pool.tile([NPM, HFREE], f32, tag=f"sc{h}")
                        nc.vector.tensor_scalar_mul(sc[:], pre_cur[:], 0.9)
                        nc.vector.tensor_tensor(
                            pre_nxt[:], sc[:], ps[0:NPM, 0:HFREE],
                            op=mybir.AluOpType.add,
                        )
                        nc.scalar.activation(
                            th[:], pre_nxt[:],
                            mybir.ActivationFunctionType.Tanh,
                        )
                        nc.vector.tensor_tensor(
                            rd_nxt[0:NPM, :], th[:], zeros[:],
                            op=mybir.AluOpType.max,
                        )
                        if t > 0:
                            # y_{t-1} of this half -> ytmp (+bias)
                            nc.vector.tensor_scalar_add(
                                ytmp[:, h * HB : (h + 1) * HB],
                                ps[0:OUT, HFREE : HFREE + HB],
                                fcb[:],
                            )
                        if t + 2 < n_steps:
                            stage_data(t + 2, h, rd)
                    if t > 0:
                        encode_y(s, ytmp)
                        if s % ch == ch - 1:
                            c0 = s - ch + 1
                            nc.sync.dma_start(
                                y_ap[:, (c0 // YGRP) * YGB * BS
                                     : (c0 // YGRP) * YGB * BS
                                     + (ch // YGRP) * YGB * BS],
                                ybuf[:],
                            )
                            nc.sync.dma_start(sc_ap[:, c0 : c0 + ch], scbuf[:])

                # trailing: y of the last step, per half
                s = n_steps - 1
                if s % ch == 0:
                    ybuf = yout_pool.tile(
                        [OUT, (ch // YGRP) * YGB * BS], i8, tag="ybuf"
                    )
                    scbuf = yout_pool.tile([OUT, ch], bf16, tag="scbuf")
                ytmp = tmp_pool.tile([OUT, BS], f32, tag="ytmp")
                for h in range(NH):
                    ps = ps_pool.tile([128, 512], f32, tag=f"ps{h}")
                    nc.tensor.matmul(
                        ps[0:OUT, HFREE : HFREE + HB],
                        fct[:],
                        rds[h][n_steps % 2][0:KJ, 0:HB],
                        start=True,
                        stop=True,
                    )
                    nc.vector.tensor_scalar_add(
                        ytmp[:, h * HB : (h + 1) * HB],
                        ps[0:OUT, HFREE : HFREE + HB],
                        fcb[:],
                    )
                encode_y(s, ytmp)
                c0 = s - s % ch
                ng0 = c0 // YGRP
                nglast = _ygroups(n_steps) - ng0
                nc.sync.dma_start(
                    y_ap[:, ng0 * YGB * BS : (ng0 + nglast) * YGB * BS],
                    ybuf[:, : nglast * YGB * BS],
                )
                nc.sync.dma_start(
                    sc_ap[:, c0 : n_steps], scbuf[:, : n_steps - c0]
                )

    nc.compile()
    return nc


def _get_program(n_steps: int, n_repeat: int = 1):
    key = (n_steps, Y_BITS, D_CUBIC_W, W_MODE, n_repeat)
    if key not in _BUILD_CACHE:
        _BUILD_CACHE[key] = _build_program(n_steps, n_repeat)
    return _BUILD_CACHE[key]


def _plan_up_chunks(n_steps: int):
    k = min(N_UP_CHUNKS, n_steps)
    base = n_steps // k
    rem = n_steps - base * k
    return [base + (1 if i < rem else 0) for i in range(k)]


def _make_encode_lut(amax: float):
    """uint16-indexed LUT: i = rint(x*32767/amax)+32767 -> int8 level index.

    Built by exact inversion on the 65535 bin centers (cheap), so the
    per-element encode is one mul+rint+gather.
    """
    w = D_CUBIC_W
    # index j corresponds to i = j - 32768 (j = int16(i) ^ 0x8000)
    xs = ((np.arange(65536, dtype=np.float64) - 32768.0) / 32767.0) * amax
    if w <= 0:
        u = xs / amax
    else:
        # solve w*u + (1-w)*u^3 = x/amax by Cardano (monotone odd cubic)
        a = 1.0 - w
        p = w / a
        qq = -np.abs(xs) / (amax * a)
        disc = np.sqrt((qq * qq) / 4.0 + (p ** 3) / 27.0)
        u = np.cbrt(-qq / 2.0 + disc) + np.cbrt(-qq / 2.0 - disc)
        u = np.copysign(u, xs)
    q = np.rint(np.clip(u, -1.0, 1.0) * 127.0).astype(np.int8)
    return q


def _cubic_encode(data_f: np.ndarray, amax: float, lut):
    i = np.rint(data_f * np.float32(32767.0 / amax)).astype(np.int16)
    return lut[i.view(np.uint16).ravel() ^ 0x8000].reshape(data_f.shape)


def _prep_weights(J, I, S, Bb, x0, fc_w, fc_b, amax):
    f32 = np.float32
    dsc = 1.0  # data scale rides in the compander constants, not weights
    Jp = 0.1 * np.asarray(J, f32)
    Ip = 0.1 * np.asarray(I, f32)
    Sp = 0.1 * np.asarray(S, f32)
    Bbp = 0.1 * np.asarray(Bb, f32)

    jt = np.zeros((KJ, 9, NPM), f32)
    for k in range(NMOD):
        for m in range(NMOD):
            blk = Jp[m * NPM : (m + 1) * NPM, k * NPM : (k + 1) * NPM]
            jt[:NPM, k * NMOD + m, :NPM] = blk.T
            if k == 0:
                jt[NPM : NPM + NF, k * NMOD + m, :NPM] = (
                    Ip[m * NPM : (m + 1) * NPM, :].T
                )
                jt[NPM + NF, k * NMOD + m, :NPM] = Sp[m * NPM : (m + 1) * NPM, 0]
                jt[NPM + NF + 1, k * NMOD + m, :NPM] = (
                    Bbp[m * NPM : (m + 1) * NPM, 0]
                )

    wall = np.zeros((KJ, WALL_COLS), f32)
    wall[:, : 9 * NPM] = jt.reshape(KJ, 9 * NPM)
    wall[:NPM, WALL_FCT : WALL_FCT + OUT] = np.asarray(fc_w, f32).T
    wall[0, WALL_ONES : WALL_ONES + HB] = 1.0
    # compander decode constants per data row: x = c1*q + c3*q^3
    w = D_CUBIC_W
    if w > 0:
        c1 = amax * w / 127.0
        c3 = amax * (1.0 - w) / (127.0 ** 3)
    else:
        c1 = amax / 127.0
        c3 = 0.0
    wall[:KDATA, WALL_ONES + HB] = c1
    wall[:KDATA, WALL_ONES + HB + 1] = c3
    wall[:OUT, WALL_FCB] = np.asarray(fc_b, f32)
    wall[:NPM, WALL_X0 : WALL_X0 + NMOD] = (
        np.asarray(x0, f32).reshape(NMOD, NPM).T
    )
    if W_MODE == "gather":
        # core c uploads column block c: global [8*KJ, WBLK]
        return np.ascontiguousarray(
            wall.reshape(KJ, N_CORES, WBLK).transpose(1, 0, 2)
        ).reshape(N_CORES * KJ, WBLK)
    return np.ascontiguousarray(
        np.broadcast_to(wall, (N_CORES, KJ, WALL_COLS))
    ).reshape(N_CORES * KJ, WALL_COLS)


def _prep_din_chunk(dat_f, t0, c):
    """[T,21,B] f32 -> companded int8 [8*21, c*BS], core-major."""
    amax = _prep_din_chunk.amax
    q = _cubic_encode(dat_f[t0 : t0 + c], amax, _prep_din_chunk.lut)
    return np.ascontiguousarray(
        q.reshape(c, KDATA, N_CORES, BS).transpose(2, 1, 0, 3)
    ).reshape(N_CORES * KDATA, c * BS)


class _Runner:
    """Persistent jitted shard_map callable for one compiled program."""

    def __init__(self, nc):
        import jax
        import jax.numpy as jnp
        from jax.sharding import Mesh, PartitionSpec
        from jax.experimental.shard_map import shard_map
        from concourse.bass2jax import (
            _bass_exec_p,
            install_neuronx_cc_hook,
            partition_id_tensor,
        )

        install_neuronx_cc_hook()
        self.nc = nc
        partition_name = (
            nc.partition_id_tensor.name if nc.partition_id_tensor else None
        )

        in_names, out_names, out_avals, zero_shapes = [], [], [], []
        for alloc in nc.m.functions[0].allocations:
            if not isinstance(alloc, mybir.MemoryLocationSet):
                continue
            name = alloc.memorylocations[0].name
            if alloc.kind == "ExternalInput":
                if name != partition_name:
                    in_names.append(name)
            elif alloc.kind == "ExternalOutput":
                np_dt = mybir.dt.np(alloc.dtype)
                out_avals.append(
                    jax.core.ShapedArray(tuple(alloc.tensor_shape), np_dt)
                )
                out_names.append(name)
                zero_shapes.append((tuple(alloc.tensor_shape), np_dt))
        assert out_names[0] == "y", out_names
        self.in_names = in_names
        self.out_names = out_names

        n_params = len(in_names)
        n_outs = len(out_names)
        all_in_names = list(in_names) + list(out_names)
        if partition_name is not None:
            all_in_names.append(partition_name)

        def _body(*args):
            operands = list(args)
            if partition_name is not None:
                operands.append(partition_id_tensor())
            outs = _bass_exec_p.bind(
                *operands,
                out_avals=tuple(out_avals),
                in_names=tuple(all_in_names),
                out_names=tuple(out_names),
                lowering_input_output_aliases=(),
                sim_require_finite=True,
                sim_require_nnan=True,
                nc=nc,
            )
            return tuple(outs)

        devices = jax.devices()[:N_CORES]
        mesh = Mesh(np.asarray(devices), ("core",))
        in_specs = (PartitionSpec("core"),) * (n_params + n_outs)
        out_specs = (PartitionSpec("core"),) * n_outs
        self.mesh = mesh
        self.spec = jax.sharding.NamedSharding(mesh, PartitionSpec("core"))
        self.sharded = jax.jit(
            shard_map(
                _body, mesh=mesh, in_specs=in_specs, out_specs=out_specs,
                check_rep=False,
            ),
            keep_unused=True,
        )
        self.zeros = [
            jnp.zeros((N_CORES * shp[0], *shp[1:]), dt)
            for shp, dt in zero_shapes
        ]

    def __call__(self, *ins):
        return self.sharded(*ins, *self.zeros)


def _get_runner(n_steps: int, n_repeat: int = 1):
    key = (n_steps, Y_BITS, D_CUBIC_W, W_MODE, n_repeat)
    if key not in _RUNNER_CACHE:
        _RUNNER_CACHE[key] = _Runner(_get_program(n_steps, n_repeat))
    return _RUNNER_CACHE[key]


def _decode_shard(final, ybytes, scales, c, n_steps):
    """6/5-bit DPCM shard -> final[:, c*BS:(c+1)*BS, :] f32."""
    ng = _ygroups(n_steps)
    # device wrote (byte - 128) as int8; uint8 view of that is byte XOR 128
    bb = (ybytes.view(np.uint8) ^ 128).astype(np.int16)
    bb = bb.reshape(OUT, ng, YGB, BS)
    v = np.empty((OUT, ng, YGRP, BS), np.int16)
    b = [bb[:, :, k] for k in range(YGB)]
    if Y_BITS == 6:
        v[:, :, 0] = b[0] >> 2
        v[:, :, 1] = ((b[0] & 3) << 4) | (b[1] >> 4)
        v[:, :, 2] = ((b[1] & 15) << 2) | (b[2] >> 6)
        v[:, :, 3] = b[2] & 63
    else:
        v[:, :, 0] = b[0] >> 3
        v[:, :, 1] = ((b[0] & 7) << 2) | (b[1] >> 6)
        v[:, :, 2] = (b[1] >> 1) & 31
        v[:, :, 3] = ((b[1] & 1) << 4) | (b[2] >> 4)
        v[:, :, 4] = ((b[2] & 15) << 1) | (b[3] >> 7)
        v[:, :, 5] = (b[3] >> 2) & 31
        v[:, :, 6] = ((b[3] & 3) << 3) | (b[4] >> 5)
        v[:, :, 7] = b[4] & 31
    v = v.reshape(OUT, ng * YGRP, BS)[:, :n_steps]
    # y_t = cumsum_t(q*s) = cumsum(v*s) - Q_OFF*cumsum(s)
    s = scales.astype(np.float32)[:, :, None]        # [OUT, T, 1]
    ys = v.astype(np.float32)
    ys *= s
    np.cumsum(ys, axis=1, out=ys)
    cs = np.cumsum(s, axis=1, dtype=np.float32)
    ys -= Q_OFF * cs
    final[:, c * BS : (c + 1) * BS, :] = ys.transpose(1, 2, 0)


def run_sharded(inputs: dict, n_steps: int = T):
    """Compile (cached), run on 8 cores, return [T, B, OUT] f32."""
    import jax
    import os
    import time as _time
    from concurrent.futures import ThreadPoolExecutor

    dbg = os.environ.get("K_DEBUG_TIMING")
    tt0 = _time.time()

    def mark(lbl):
        if dbg:
            print(f"  [{(_time.time()-tt0)*1e3:7.1f}ms] {lbl}", flush=True)

    runner = _get_runner(n_steps)
    spec = runner.spec

    data = np.asarray(inputs["data"], np.float32)[:n_steps]
    amax = max(float(data.max()), -float(data.min()), 1e-30)
    _prep_din_chunk.amax = amax
    _prep_din_chunk.lut = _make_encode_lut(amax)
    mark("amax+lut")

    # async upload pipeline: issue device_puts in order WITHOUT blocking
    # (transfers ride the wire FIFO), then dispatch immediately -- the
    # ~85ms of jit/shard_map dispatch overhead and the exec (~3ms)
    # overlap the upload wire time; exec starts when the last input
    # buffer lands.
    wglob = _prep_weights(
        inputs["J"], inputs["I"], inputs["S"], inputs["Bb"],
        inputs["x0"], inputs["fc_w"], inputs["fc_b"], amax,
    )
    dev_in = {"w": jax.device_put(wglob, spec)}
    mark("w prepped+queued")
    chunks = _plan_up_chunks(n_steps)
    t0 = 0
    for i, c in enumerate(chunks):
        arr = _prep_din_chunk(data, t0, c)
        dev_in[f"din{i}"] = jax.device_put(arr, spec)
        mark(f"din{i} prepped+queued")
        t0 += c

    outs = runner(*[dev_in[n] for n in runner.in_names])
    y_g, sc_g = outs[0], outs[1]

    final = np.empty((n_steps, B, OUT), np.float32)
    ysh = sorted(y_g.addressable_shards, key=lambda s: s.index[0].start)
    ssh = sorted(sc_g.addressable_shards, key=lambda s: s.index[0].start)
    # staged fetch: keep ~2 y shards in flight so the wire stays busy
    # while the main thread decodes the shard that just landed (decode
    # ~26ms < ~55ms shard wire time, so decode fully hides).
    INFLIGHT = 2
    for c in range(min(INFLIGHT, N_CORES)):
        ysh[c].data.copy_to_host_async()
        ssh[c].data.copy_to_host_async()
    mark("exec dispatched, first fetches queued")
    for c in range(N_CORES):
        yb = np.asarray(ysh[c].data)
        scb = np.asarray(ssh[c].data)
        if c + INFLIGHT < N_CORES:
            ysh[c + INFLIGHT].data.copy_to_host_async()
            ssh[c + INFLIGHT].data.copy_to_host_async()
        mark(f"shard {c} fetched")
        _decode_shard(final, yb, scb, c, n_steps)
    mark("decoded")
    return final


def kernel(data, J, I, S, Bb, x0, fc_w, fc_b):
    return run_sharded(
        dict(data=data, J=J, I=I, S=S, Bb=Bb, x0=x0, fc_w=fc_w, fc_b=fc_b)
    )
